# revision 34
# baseline (speedup 1.0000x reference)
"""Trainium2 Bass kernel for nn_LstmModel (TF-style LSTM, T=256, F=64, H=32,
dense(1)+ELU head), data-parallel over 8 NeuronCores.

v2 design (per core, B_loc = 2048 rows):
  - x is transposed + cast to fp16 on host: no on-chip transpose pass.
  - 2 independent batch streams of 1024 rows each; per stream the state is
    chunk-packed [128 = 4 subchunks x 32 h, 256 batch].
  - gates PSUM tile per stream [128, 4 gates, 256] (2 banks), double-buffered.
  - per step per stream:
      PE: rank-1 bias matmul seeds the f-gate slice with (b_f + 1); X-pass is
          4 gates x 2 col-tiled matmuls (rhs [128,256] = 2 subchunks); H-pass
          is 4 block-diagonal full-array matmuls vs h [128,256].
      ACT: one Sigmoid over the whole [128, 1024] gate tile (j cols pre-scaled
          by 2 so tanh(j) = 2*sig(2j)-1), one Tanh over c (merged across
          streams, [128,512]).
      DVE: u=(sig2j-0.5)*sigi; v=c*f'; c=2u+v; h=tanh_c*o'   (all fp16 SBUF)
  - tail: block-diag dense matmul + ELU per stream.
"""

import os
import sys

import numpy as np

sys.path.insert(0, "/opt/trn_rl_repo")

# ---- problem constants (hardcoded per harness contract) ----
B_FULL = 16384
T = 256
F = 64
H = 32
FORGET_BIAS = 1.0
N_CORES = 8
B_LOC = B_FULL // N_CORES          # 2048
N_STREAM = 2
SB = 256                           # batch per subchunk (free dim)
T_BLK = 16                         # time steps per x DMA block
N_BLK = T // T_BLK                 # 16 blocks

_CACHE = {}


def _build_kernel(b_lstm_host, bd_val):
    import concourse.bass as bass  # noqa: F401
    import concourse.tile as tile
    from concourse import bacc, mybir

    f32 = mybir.dt.float32
    f16 = mybir.dt.float16
    AF = mybir.ActivationFunctionType
    OP = mybir.AluOpType

    b = b_lstm_host.astype(np.float32)
    # bank order [j, i, f, o]; reference gate order is i, j, f, o
    b_g = [b[32 * g:32 * g + 32].copy() for g in (1, 0, 2, 3)]
    b_g[2] += FORGET_BIAS
    need_bias = [bool(np.any(b_g[g] != 0.0)) for g in range(4)]

    nc = bacc.Bacc(None, target_bir_lowering=False, debug=False)

    with tile.TileContext(nc) as tc:
        with tc.tile_pool(name="dram", bufs=1, space="DRAM") as dram:
            # x pre-arranged on host to [p=64a+f, tb, ti, j, s, b]
            x_in = dram.tile([128, N_BLK, T_BLK, 2, N_STREAM, SB], f16,
                             kind="ExternalInput", name="x_in", uniquify=False)
            wx_in = dram.tile([128, 4, 128], f16, kind="ExternalInput",
                              name="wx_in", uniquify=False)
            wh_in = dram.tile([128, 4, 128], f16, kind="ExternalInput",
                              name="wh_in", uniquify=False)
            bias_in = dram.tile([1, 4, 128], f16, kind="ExternalInput",
                                name="bias_in", uniquify=False)
            ones_in = dram.tile([1, N_STREAM * SB], f16, kind="ExternalInput",
                                name="ones_in", uniquify=False)
            wd_in = dram.tile([128, 4], f16, kind="ExternalInput",
                              name="wd_in", uniquify=False)
            out_ext = dram.tile([N_STREAM, 4, SB], f32, kind="ExternalOutput",
                                name="out_ext", uniquify=False)

            from contextlib import ExitStack
            stk = ExitStack()
            const = stk.enter_context(tc.tile_pool(name="const", bufs=1))
            wx = const.tile([128, 4, 128], f16)
            wh = const.tile([128, 4, 128], f16)
            bias_t = const.tile([1, 4, 128], f16)
            onesr = const.tile([1, N_STREAM * SB], f16)
            wd = const.tile([128, 4], f16)
            nc.sync.dma_start(out=wx[:], in_=wx_in[:])
            nc.sync.dma_start(out=wh[:], in_=wh_in[:])
            nc.sync.dma_start(out=bias_t[:], in_=bias_in[:])
            nc.sync.dma_start(out=onesr[:], in_=ones_in[:])
            nc.sync.dma_start(out=wd[:], in_=wd_in[:])

            # persistent state (both streams side by side where useful)
            state = stk.enter_context(tc.tile_pool(name="state", bufs=1))
            c_st = state.tile([128, N_STREAM, SB], f16)
            tanh_c = [state.tile([128, SB], f16, name=f"tanh_c{s}")
                      for s in range(N_STREAM)]
            h_st = [state.tile([128, SB], f16, name=f"h_st{s}")
                    for s in range(N_STREAM)]
            # bank order is [j, i, f, o]; tanh(j) lands in TJ, sigmoids in S
            TJ = [state.tile([128, SB], f16, name=f"TJ{s}")
                  for s in range(N_STREAM)]
            S = [state.tile([128, 3, SB], f16, name=f"S{s}")
                 for s in range(N_STREAM)]
            u_t = [state.tile([128, SB], f16, name=f"u_t{s}")
                   for s in range(N_STREAM)]
            v_t = [state.tile([128, SB], f16, name=f"v_t{s}")
                   for s in range(N_STREAM)]

            nc.vector.memset(c_st[:], 0.0)

            psum_p = stk.enter_context(
                tc.tile_pool(name="psp", bufs=1, space="PSUM"))
            # parity-alternated gate tiles, both streams side by side in the
            # free dim: [128 = 4 subchunks x 32h, 4 gates (1 bank each),
            # 512 = 2 streams x 256 batch]
            ps_par = [psum_p.tile([128, 4, N_STREAM * SB], f32,
                                  name=f"ps_par{p}") for p in range(2)]
            xpool = stk.enter_context(tc.tile_pool(name="xpool", bufs=2))

            def x_block(t, xblk, gates):
                """bias + X-pass rect matmuls for both streams (N=512)."""
                ps = ps_par[t % 2]
                ti = t % T_BLK
                last_x = (t == 0)  # no H contribution at t=0
                for g in gates:
                    if need_bias[g]:
                        nc.tensor.matmul(
                            ps[:, g, :], bias_t[0:1, g, :], onesr[0:1, :],
                            start=True, stop=False,
                            tile_position=(0, 0), skip_group_check=True)
                    for q in range(4):
                        a, j = q % 2, q // 2
                        nc.tensor.matmul(
                            ps[32 * q:32 * q + 32, g, :],
                            wx[64 * a:64 * a + 64, g, 32 * q:32 * q + 32],
                            xblk[64 * a:64 * a + 64, ti, j, :, :],
                            start=not need_bias[g], stop=last_x,
                            tile_position=(64 * a, 32 * q),
                            skip_group_check=True)

            def h_block(t, s):
                ps = ps_par[t % 2]
                for g in range(4):
                    for q in range(4):
                        nc.tensor.matmul(
                            ps[32 * q:32 * q + 32, g, SB * s:SB * s + SB],
                            wh[32 * q:32 * q + 32, g, 32 * q:32 * q + 32],
                            h_st[s][32 * q:32 * q + 32, :],
                            start=False, stop=(s == N_STREAM - 1),
                            tile_position=(32 * q, 32 * q),
                            skip_group_check=True)

            def cell_update(t):
                ps = ps_par[t % 2]
                for s in range(N_STREAM):
                    nc.scalar.activation(TJ[s][:], ps[:, 0, SB * s:SB * s + SB],
                                         AF.Tanh)
                    nc.scalar.activation(S[s][:],
                                         ps[:, 1:4, SB * s:SB * s + SB],
                                         AF.Sigmoid)
                for s in range(N_STREAM):
                    # u = tanh(j) * sig(i)
                    nc.vector.tensor_tensor(
                        u_t[s][:], TJ[s][:], S[s][:, 0, :], OP.mult)
                    # v = c * f'
                    nc.vector.tensor_tensor(
                        v_t[s][:], c_st[:, s, :], S[s][:, 1, :], OP.mult)
                    # c = u + v
                    nc.vector.tensor_tensor(
                        c_st[:, s, :], u_t[s][:], v_t[s][:], OP.add)
                    nc.scalar.activation(tanh_c[s][:], c_st[:, s, :], AF.Tanh)
                for s in range(N_STREAM):
                    nc.vector.tensor_tensor(
                        h_st[s][:], tanh_c[s][:], S[s][:, 2, :], OP.mult)

            xblks = []
            for tb in range(N_BLK):
                xblk = xpool.tile([128, T_BLK, 2, N_STREAM, SB], f16,
                                  tag="xblk")
                nc.sync.dma_start(out=xblk[:], in_=x_in[:, tb])
                xblks.append(xblk)

            # PE order per step: H_A, X(t+1) half, H_B, X(t+1) rest — keeps
            # the array busy while the cell-update chain produces h
            x_block(0, xblks[0], [0, 1, 2, 3])
            for t in range(T):
                if t > 0:
                    h_block(t, 0)
                if t + 1 < T:
                    x_block(t + 1, xblks[(t + 1) // T_BLK], [0, 1])
                if t > 0:
                    h_block(t, 1)
                if t + 1 < T:
                    x_block(t + 1, xblks[(t + 1) // T_BLK], [2, 3])
                cell_update(t)

            # ---- dense head + ELU per stream ----
            for s in range(N_STREAM):
                y_ps = ps_par[s][0:4, 0, 0:SB]
                nc.tensor.matmul(y_ps, wd[:], h_st[s][:],
                                 start=True, stop=True,
                                 tile_position=(0, 0), skip_group_check=True)
                ybd = state.tile([4, SB], f32)
                m0 = state.tile([4, SB], f32)
                ex = state.tile([4, SB], f32)
                elu = state.tile([4, SB], f32)
                nc.vector.tensor_scalar_add(ybd[:], y_ps, float(bd_val))
                nc.vector.tensor_scalar_min(m0[:], ybd[:], 0.0)
                nc.scalar.activation(ex[:], m0[:], AF.Exp)
                nc.vector.scalar_tensor_tensor(
                    elu[:], ex[:], 1.0, ybd[:], OP.subtract, OP.max)
                nc.sync.dma_start(out=out_ext[s], in_=elu[:])
            stk.close()

    nc.compile()
    return nc


def _prep_weights(W_lstm, b_lstm, W_dense, b_dense):
    Wx = W_lstm[:F, :].astype(np.float32).copy()   # [64, 128]
    Wh = W_lstm[F:, :].astype(np.float32).copy()   # [32, 128]

    # bank order [j, i, f, o] (ref col order i, j, f, o)
    bank_src = (1, 0, 2, 3)
    wx_host = np.zeros((128, 4, 128), np.float32)
    wh_host = np.zeros((128, 4, 128), np.float32)
    for g in range(4):
        gs = bank_src[g]
        for q in range(4):
            a = q % 2
            wx_host[64 * a:64 * a + 64, g, 32 * q:32 * q + 32] = \
                Wx[:, 32 * gs:32 * gs + 32]
            wh_host[32 * q:32 * q + 32, g, 32 * q:32 * q + 32] = \
                Wh[:, 32 * gs:32 * gs + 32]

    b = b_lstm.astype(np.float32).copy()
    b_g = [b[32 * g:32 * g + 32].copy() for g in bank_src]
    b_g[2] += FORGET_BIAS
    bias_host = np.zeros((1, 4, 128), np.float32)
    for g in range(4):
        bias_host[0, g, :] = np.tile(b_g[g], 4)

    ones_host = np.ones((1, N_STREAM * SB), np.float32)
    wd_host = np.zeros((128, 4), np.float32)
    for q in range(4):
        wd_host[32 * q:32 * q + 32, q] = W_dense[:, 0]
    bd_host = np.array([[np.float32(b_dense.reshape(-1)[0])]], np.float32)
    return (wx_host.astype(np.float16), wh_host.astype(np.float16),
            bias_host.astype(np.float16), ones_host.astype(np.float16),
            wd_host.astype(np.float16), bd_host)


def kernel(x, W_lstm, b_lstm, W_dense, b_dense):
    from concourse.bass_utils import run_bass_kernel_spmd

    x = np.asarray(x, np.float32)
    key = "k"
    if key not in _CACHE:
        _CACHE[key] = _build_kernel(
            np.asarray(b_lstm, np.float32),
            float(np.asarray(b_dense).reshape(-1)[0]))
    nc = _CACHE[key]

    wx, wh, bias_h, ones_h, wd, bd = _prep_weights(
        np.asarray(W_lstm, np.float32), np.asarray(b_lstm, np.float32),
        np.asarray(W_dense, np.float32), np.asarray(b_dense, np.float32))

    in_maps = []
    for c in range(N_CORES):
        xs = x[c * B_LOC:(c + 1) * B_LOC]  # [2048, 16384]
        # [s, j, a, b, tb, ti, f] -> [a, f, tb, ti, j, s, b]
        x7 = xs.reshape(N_STREAM, 2, 2, SB, N_BLK, T_BLK, F)
        x7 = np.ascontiguousarray(
            x7.transpose(2, 6, 4, 5, 1, 0, 3)).astype(np.float16)
        x6 = x7.reshape(128, N_BLK, T_BLK, 2, N_STREAM, SB)
        in_maps.append({
            "x_in": x6, "wx_in": wx, "wh_in": wh, "bias_in": bias_h,
            "ones_in": ones_h, "wd_in": wd,
        })

    res = run_bass_kernel_spmd(nc, in_maps, core_ids=list(range(N_CORES)),
                               tmpdir=os.environ.get("BASS_TMPDIR") or None)
    global LAST_EXEC_NS, LAST_RESULT
    LAST_EXEC_NS = res.exec_time_ns
    LAST_RESULT = res
    outs = [r["out_ext"].reshape(-1) for r in res.results]
    return np.concatenate(outs).astype(np.float32)


LAST_EXEC_NS = None
LAST_RESULT = None


# revision 38
# speedup vs baseline: 1.0777x; 1.0777x over previous
"""Trainium2 Bass kernel for nn_LstmModel (TF-style LSTM, T=256, F=64, H=32,
dense(1)+ELU head), data-parallel over 8 NeuronCores.

v2 design (per core, B_loc = 2048 rows):
  - x is transposed + cast to fp16 on host: no on-chip transpose pass.
  - 2 independent batch streams of 1024 rows each; per stream the state is
    chunk-packed [128 = 4 subchunks x 32 h, 256 batch].
  - gates PSUM tile per stream [128, 4 gates, 256] (2 banks), double-buffered.
  - per step per stream:
      PE: rank-1 bias matmul seeds the f-gate slice with (b_f + 1); X-pass is
          4 gates x 2 col-tiled matmuls (rhs [128,256] = 2 subchunks); H-pass
          is 4 block-diagonal full-array matmuls vs h [128,256].
      ACT: one Sigmoid over the whole [128, 1024] gate tile (j cols pre-scaled
          by 2 so tanh(j) = 2*sig(2j)-1), one Tanh over c (merged across
          streams, [128,512]).
      DVE: u=(sig2j-0.5)*sigi; v=c*f'; c=2u+v; h=tanh_c*o'   (all fp16 SBUF)
  - tail: block-diag dense matmul + ELU per stream.
"""

import os
import sys

import numpy as np

sys.path.insert(0, "/opt/trn_rl_repo")

# ---- problem constants (hardcoded per harness contract) ----
B_FULL = 16384
T = 256
F = 64
H = 32
FORGET_BIAS = 1.0
N_CORES = 8
B_LOC = B_FULL // N_CORES          # 2048
N_STREAM = 2
SB = 256                           # batch per subchunk (free dim)
T_BLK = 16                         # time steps per x DMA block
N_BLK = T // T_BLK                 # 16 blocks

_CACHE = {}


def _build_kernel(b_lstm_host, bd_val):
    import concourse.bass as bass  # noqa: F401
    import concourse.tile as tile
    from concourse import bacc, mybir

    f32 = mybir.dt.float32
    f16 = mybir.dt.float16
    AF = mybir.ActivationFunctionType
    OP = mybir.AluOpType

    b = b_lstm_host.astype(np.float32)
    # bank order [j, i, f, o]; reference gate order is i, j, f, o
    b_g = [b[32 * g:32 * g + 32].copy() for g in (1, 0, 2, 3)]
    b_g[0] *= 2.0
    b_g[2] += FORGET_BIAS
    need_bias = [bool(np.any(b_g[g] != 0.0)) for g in range(4)]

    nc = bacc.Bacc(None, target_bir_lowering=False, debug=False)

    with tile.TileContext(nc) as tc:
        with tc.tile_pool(name="dram", bufs=1, space="DRAM") as dram:
            # x pre-arranged on host to [p=64a+f, tb, ti, j, s, b]
            x_in = dram.tile([128, N_BLK, T_BLK, 2, N_STREAM, SB], f16,
                             kind="ExternalInput", name="x_in", uniquify=False)
            wx_in = dram.tile([128, 4, 128], f16, kind="ExternalInput",
                              name="wx_in", uniquify=False)
            wh_in = dram.tile([128, 4, 128], f16, kind="ExternalInput",
                              name="wh_in", uniquify=False)
            bias_in = dram.tile([1, 4, 128], f16, kind="ExternalInput",
                                name="bias_in", uniquify=False)
            ones_in = dram.tile([1, N_STREAM * SB], f16, kind="ExternalInput",
                                name="ones_in", uniquify=False)
            wd_in = dram.tile([128, 4], f16, kind="ExternalInput",
                              name="wd_in", uniquify=False)
            out_ext = dram.tile([N_STREAM, 4, SB], f32, kind="ExternalOutput",
                                name="out_ext", uniquify=False)

            from contextlib import ExitStack
            stk = ExitStack()
            const = stk.enter_context(tc.tile_pool(name="const", bufs=1))
            wx = const.tile([128, 4, 128], f16)
            wh = const.tile([128, 4, 128], f16)
            bias_t = const.tile([1, 4, 128], f16)
            onesr = const.tile([1, N_STREAM * SB], f16)
            wd = const.tile([128, 4], f16)
            nc.sync.dma_start(out=wx[:], in_=wx_in[:])
            nc.sync.dma_start(out=wh[:], in_=wh_in[:])
            nc.sync.dma_start(out=bias_t[:], in_=bias_in[:])
            nc.sync.dma_start(out=onesr[:], in_=ones_in[:])
            nc.sync.dma_start(out=wd[:], in_=wd_in[:])

            # persistent state (both streams side by side where useful)
            state = stk.enter_context(tc.tile_pool(name="state", bufs=1))
            c_st = state.tile([128, N_STREAM, SB], f16)
            tanh_c = [state.tile([128, SB], f16, name=f"tanh_c{s}")
                      for s in range(N_STREAM)]
            h_st = [state.tile([128, SB], f16, name=f"h_st{s}")
                    for s in range(N_STREAM)]
            # bank order is [j, i, f, o]; S holds sig over all four banks
            TJ = [state.tile([128, SB], f16, name=f"TJ{s}")
                  for s in range(N_STREAM)]
            S = [state.tile([128, 4, SB], f16, name=f"S{s}")
                 for s in range(N_STREAM)]
            u_t = [state.tile([128, SB], f16, name=f"u_t{s}")
                   for s in range(N_STREAM)]
            v_t = [state.tile([128, SB], f16, name=f"v_t{s}")
                   for s in range(N_STREAM)]

            nc.vector.memset(c_st[:], 0.0)

            psum_p = stk.enter_context(
                tc.tile_pool(name="psp", bufs=1, space="PSUM"))
            # parity-alternated gate tiles, both streams side by side in the
            # free dim: [128 = 4 subchunks x 32h, 4 gates (1 bank each),
            # 512 = 2 streams x 256 batch]
            ps_par = [psum_p.tile([128, 4, N_STREAM * SB], f32,
                                  name=f"ps_par{p}") for p in range(2)]
            xpool = stk.enter_context(tc.tile_pool(name="xpool", bufs=2))

            def x_block(t, xblk, gates):
                """bias + X-pass rect matmuls for both streams (N=512)."""
                ps = ps_par[t % 2]
                ti = t % T_BLK
                last_x = (t == 0)  # no H contribution at t=0
                for g in gates:
                    if need_bias[g]:
                        nc.tensor.matmul(
                            ps[:, g, :], bias_t[0:1, g, :], onesr[0:1, :],
                            start=True, stop=False,
                            tile_position=(0, 0), skip_group_check=True)
                    for q in range(4):
                        a, j = q % 2, q // 2
                        nc.tensor.matmul(
                            ps[32 * q:32 * q + 32, g, :],
                            wx[64 * a:64 * a + 64, g, 32 * q:32 * q + 32],
                            xblk[64 * a:64 * a + 64, ti, j, :, :],
                            start=not need_bias[g], stop=last_x,
                            tile_position=(64 * a, 32 * q),
                            skip_group_check=True)

            def h_block(t, s):
                ps = ps_par[t % 2]
                for g in range(4):
                    for q in range(4):
                        nc.tensor.matmul(
                            ps[32 * q:32 * q + 32, g, SB * s:SB * s + SB],
                            wh[32 * q:32 * q + 32, g, 32 * q:32 * q + 32],
                            h_st[s][32 * q:32 * q + 32, :],
                            start=False, stop=(s == N_STREAM - 1),
                            tile_position=(32 * q, 32 * q),
                            skip_group_check=True)

            def cell_update(t):
                ps = ps_par[t % 2]
                for s in range(N_STREAM):
                    nc.scalar.activation(S[s][:],
                                         ps[:, :, SB * s:SB * s + SB],
                                         AF.Sigmoid)
                for s in range(N_STREAM):
                    # tj = 2*sig(2j) - 1 = tanh(j)   (4x-mode tensor_scalar)
                    nc.vector.tensor_scalar(
                        out=TJ[s][:], in0=S[s][:, 0, :],
                        scalar1=2.0, scalar2=-1.0, op0=OP.mult, op1=OP.add)
                    # u = tanh(j) * sig(i)
                    nc.vector.tensor_tensor(
                        u_t[s][:], TJ[s][:], S[s][:, 1, :], OP.mult)
                    # v = c * f'
                    nc.vector.tensor_tensor(
                        v_t[s][:], c_st[:, s, :], S[s][:, 2, :], OP.mult)
                    # c = u + v
                    nc.vector.tensor_tensor(
                        c_st[:, s, :], u_t[s][:], v_t[s][:], OP.add)
                    nc.scalar.activation(tanh_c[s][:], c_st[:, s, :], AF.Tanh)
                for s in range(N_STREAM):
                    nc.vector.tensor_tensor(
                        h_st[s][:], tanh_c[s][:], S[s][:, 3, :], OP.mult)

            xblks = []
            for tb in range(N_BLK):
                xblk = xpool.tile([128, T_BLK, 2, N_STREAM, SB], f16,
                                  tag="xblk")
                nc.sync.dma_start(out=xblk[:], in_=x_in[:, tb])
                xblks.append(xblk)

            # PE order per step: H_A, X(t+1) half, H_B, X(t+1) rest — keeps
            # the array busy while the cell-update chain produces h
            x_block(0, xblks[0], [0, 1, 2, 3])
            for t in range(T):
                if t > 0:
                    h_block(t, 0)
                if t + 1 < T:
                    x_block(t + 1, xblks[(t + 1) // T_BLK], [0, 1])
                if t > 0:
                    h_block(t, 1)
                if t + 1 < T:
                    x_block(t + 1, xblks[(t + 1) // T_BLK], [2, 3])
                cell_update(t)

            # ---- dense head + ELU per stream ----
            for s in range(N_STREAM):
                y_ps = ps_par[s][0:4, 0, 0:SB]
                nc.tensor.matmul(y_ps, wd[:], h_st[s][:],
                                 start=True, stop=True,
                                 tile_position=(0, 0), skip_group_check=True)
                ybd = state.tile([4, SB], f32)
                m0 = state.tile([4, SB], f32)
                ex = state.tile([4, SB], f32)
                elu = state.tile([4, SB], f32)
                nc.vector.tensor_scalar_add(ybd[:], y_ps, float(bd_val))
                nc.vector.tensor_scalar_min(m0[:], ybd[:], 0.0)
                nc.scalar.activation(ex[:], m0[:], AF.Exp)
                nc.vector.scalar_tensor_tensor(
                    elu[:], ex[:], 1.0, ybd[:], OP.subtract, OP.max)
                nc.sync.dma_start(out=out_ext[s], in_=elu[:])
            stk.close()

    nc.compile()
    return nc


def _prep_weights(W_lstm, b_lstm, W_dense, b_dense):
    Wx = W_lstm[:F, :].astype(np.float32).copy()   # [64, 128]
    Wh = W_lstm[F:, :].astype(np.float32).copy()   # [32, 128]

    # bank order [j, i, f, o] (ref col order i, j, f, o); j folded by 2 so
    # tanh(j) = 2*sig(2j) - 1
    bank_src = (1, 0, 2, 3)
    Wx[:, 32:64] *= 2.0
    Wh[:, 32:64] *= 2.0
    wx_host = np.zeros((128, 4, 128), np.float32)
    wh_host = np.zeros((128, 4, 128), np.float32)
    for g in range(4):
        gs = bank_src[g]
        for q in range(4):
            a = q % 2
            wx_host[64 * a:64 * a + 64, g, 32 * q:32 * q + 32] = \
                Wx[:, 32 * gs:32 * gs + 32]
            wh_host[32 * q:32 * q + 32, g, 32 * q:32 * q + 32] = \
                Wh[:, 32 * gs:32 * gs + 32]

    b = b_lstm.astype(np.float32).copy()
    b_g = [b[32 * g:32 * g + 32].copy() for g in bank_src]
    b_g[0] *= 2.0
    b_g[2] += FORGET_BIAS
    bias_host = np.zeros((1, 4, 128), np.float32)
    for g in range(4):
        bias_host[0, g, :] = np.tile(b_g[g], 4)

    ones_host = np.ones((1, N_STREAM * SB), np.float32)
    wd_host = np.zeros((128, 4), np.float32)
    for q in range(4):
        wd_host[32 * q:32 * q + 32, q] = W_dense[:, 0]
    bd_host = np.array([[np.float32(b_dense.reshape(-1)[0])]], np.float32)
    return (wx_host.astype(np.float16), wh_host.astype(np.float16),
            bias_host.astype(np.float16), ones_host.astype(np.float16),
            wd_host.astype(np.float16), bd_host)


def kernel(x, W_lstm, b_lstm, W_dense, b_dense):
    from concourse.bass_utils import run_bass_kernel_spmd

    x = np.asarray(x, np.float32)
    key = "k"
    if key not in _CACHE:
        _CACHE[key] = _build_kernel(
            np.asarray(b_lstm, np.float32),
            float(np.asarray(b_dense).reshape(-1)[0]))
    nc = _CACHE[key]

    wx, wh, bias_h, ones_h, wd, bd = _prep_weights(
        np.asarray(W_lstm, np.float32), np.asarray(b_lstm, np.float32),
        np.asarray(W_dense, np.float32), np.asarray(b_dense, np.float32))

    in_maps = []
    for c in range(N_CORES):
        xs = x[c * B_LOC:(c + 1) * B_LOC]  # [2048, 16384]
        # [s, j, a, b, tb, ti, f] -> [a, f, tb, ti, j, s, b]
        x7 = xs.reshape(N_STREAM, 2, 2, SB, N_BLK, T_BLK, F)
        x7 = np.ascontiguousarray(
            x7.transpose(2, 6, 4, 5, 1, 0, 3)).astype(np.float16)
        x6 = x7.reshape(128, N_BLK, T_BLK, 2, N_STREAM, SB)
        in_maps.append({
            "x_in": x6, "wx_in": wx, "wh_in": wh, "bias_in": bias_h,
            "ones_in": ones_h, "wd_in": wd,
        })

    res = run_bass_kernel_spmd(nc, in_maps, core_ids=list(range(N_CORES)),
                               tmpdir=os.environ.get("BASS_TMPDIR") or None)
    global LAST_EXEC_NS, LAST_RESULT
    LAST_EXEC_NS = res.exec_time_ns
    LAST_RESULT = res
    outs = [r["out_ext"].reshape(-1) for r in res.results]
    return np.concatenate(outs).astype(np.float32)


LAST_EXEC_NS = None
LAST_RESULT = None


# revision 40
# speedup vs baseline: 1.1524x; 1.0693x over previous
"""Trainium2 Bass kernel for nn_LstmModel (TF-style LSTM, T=256, F=64, H=32,
dense(1)+ELU head), data-parallel over 8 NeuronCores.

v2 design (per core, B_loc = 2048 rows):
  - x is transposed + cast to fp16 on host: no on-chip transpose pass.
  - 2 independent batch streams of 1024 rows each; per stream the state is
    chunk-packed [128 = 4 subchunks x 32 h, 256 batch].
  - gates PSUM tile per stream [128, 4 gates, 256] (2 banks), double-buffered.
  - per step per stream:
      PE: rank-1 bias matmul seeds the f-gate slice with (b_f + 1); X-pass is
          4 gates x 2 col-tiled matmuls (rhs [128,256] = 2 subchunks); H-pass
          is 4 block-diagonal full-array matmuls vs h [128,256].
      ACT: one Sigmoid over the whole [128, 1024] gate tile (j cols pre-scaled
          by 2 so tanh(j) = 2*sig(2j)-1), one Tanh over c (merged across
          streams, [128,512]).
      DVE: u=(sig2j-0.5)*sigi; v=c*f'; c=2u+v; h=tanh_c*o'   (all fp16 SBUF)
  - tail: block-diag dense matmul + ELU per stream.
"""

import os
import sys

import numpy as np

sys.path.insert(0, "/opt/trn_rl_repo")

# ---- problem constants (hardcoded per harness contract) ----
B_FULL = 16384
T = 256
F = 64
H = 32
FORGET_BIAS = 1.0
N_CORES = 8
B_LOC = B_FULL // N_CORES          # 2048
N_STREAM = 2
SB = 256                           # batch per subchunk (free dim)
T_BLK = 16                         # time steps per x DMA block
N_BLK = T // T_BLK                 # 16 blocks

_CACHE = {}


def _build_kernel(b_lstm_host, bd_val):
    import concourse.bass as bass  # noqa: F401
    import concourse.tile as tile
    from concourse import bacc, mybir

    f32 = mybir.dt.float32
    f16 = mybir.dt.float16
    AF = mybir.ActivationFunctionType
    OP = mybir.AluOpType

    b = b_lstm_host.astype(np.float32)
    # bank order [j, i, f, o]; reference gate order is i, j, f, o
    b_g = [b[32 * g:32 * g + 32].copy() for g in (1, 0, 2, 3)]
    b_g[0] *= 2.0
    b_g[2] += FORGET_BIAS
    need_bias = [bool(np.any(b_g[g] != 0.0)) for g in range(4)]

    nc = bacc.Bacc(None, target_bir_lowering=False, debug=False)

    with tile.TileContext(nc) as tc:
        with tc.tile_pool(name="dram", bufs=1, space="DRAM") as dram:
            # x pre-arranged on host to [p=64a+f, tb, ti, j, s, b]
            x_in = dram.tile([128, N_BLK, T_BLK, 2, N_STREAM, SB], f16,
                             kind="ExternalInput", name="x_in", uniquify=False)
            wx_in = dram.tile([128, 4, 128], f16, kind="ExternalInput",
                              name="wx_in", uniquify=False)
            wh_in = dram.tile([128, 4, 128], f16, kind="ExternalInput",
                              name="wh_in", uniquify=False)
            bias_in = dram.tile([1, 4, 128], f16, kind="ExternalInput",
                                name="bias_in", uniquify=False)
            ones_in = dram.tile([1, N_STREAM * SB], f16, kind="ExternalInput",
                                name="ones_in", uniquify=False)
            wd_in = dram.tile([128, 4], f16, kind="ExternalInput",
                              name="wd_in", uniquify=False)
            out_ext = dram.tile([N_STREAM, 4, SB], f32, kind="ExternalOutput",
                                name="out_ext", uniquify=False)

            from contextlib import ExitStack
            stk = ExitStack()
            const = stk.enter_context(tc.tile_pool(name="const", bufs=1))
            wx = const.tile([128, 4, 128], f16)
            wh = const.tile([128, 4, 128], f16)
            bias_t = const.tile([1, 4, 128], f16)
            onesr = const.tile([1, N_STREAM * SB], f16)
            wd = const.tile([128, 4], f16)
            nc.sync.dma_start(out=wx[:], in_=wx_in[:])
            nc.sync.dma_start(out=wh[:], in_=wh_in[:])
            nc.sync.dma_start(out=bias_t[:], in_=bias_in[:])
            nc.sync.dma_start(out=onesr[:], in_=ones_in[:])
            nc.sync.dma_start(out=wd[:], in_=wd_in[:])

            # persistent state (both streams side by side where useful)
            state = stk.enter_context(tc.tile_pool(name="state", bufs=1))
            c_st = state.tile([128, N_STREAM, SB], f16)
            tanh_c = [state.tile([128, SB], f16, name=f"tanh_c{s}")
                      for s in range(N_STREAM)]
            h_st = [state.tile([128, SB], f16, name=f"h_st{s}")
                    for s in range(N_STREAM)]
            # bank order is [j, i, f, o]; S holds sig over all four banks
            TJ = [state.tile([128, SB], f16, name=f"TJ{s}")
                  for s in range(N_STREAM)]
            S = [state.tile([128, 4, SB], f16, name=f"S{s}")
                 for s in range(N_STREAM)]
            u_t = [state.tile([128, SB], f16, name=f"u_t{s}")
                   for s in range(N_STREAM)]
            v_t = [state.tile([128, SB], f16, name=f"v_t{s}")
                   for s in range(N_STREAM)]

            nc.vector.memset(c_st[:], 0.0)

            psum_p = stk.enter_context(
                tc.tile_pool(name="psp", bufs=1, space="PSUM"))
            # parity-alternated gate tiles, both streams side by side in the
            # free dim: [128 = 4 subchunks x 32h, 4 gates (1 bank each),
            # 512 = 2 streams x 256 batch]
            ps_par = [psum_p.tile([128, 4, N_STREAM * SB], f32,
                                  name=f"ps_par{p}") for p in range(2)]
            xpool = stk.enter_context(tc.tile_pool(name="xpool", bufs=2))

            def x_block(t, xblk, gates):
                """bias + X-pass rect matmuls for both streams (N=512)."""
                ps = ps_par[t % 2]
                ti = t % T_BLK
                last_x = (t == 0)  # no H contribution at t=0
                for g in gates:
                    if need_bias[g]:
                        nc.tensor.matmul(
                            ps[:, g, :], bias_t[0:1, g, :], onesr[0:1, :],
                            start=True, stop=False,
                            tile_position=(0, 0), skip_group_check=True)
                    for q in range(4):
                        a, j = q % 2, q // 2
                        nc.tensor.matmul(
                            ps[32 * q:32 * q + 32, g, :],
                            wx[64 * a:64 * a + 64, g, 32 * q:32 * q + 32],
                            xblk[64 * a:64 * a + 64, ti, j, :, :],
                            start=not need_bias[g], stop=last_x,
                            tile_position=(64 * a, 32 * q),
                            skip_group_check=True)

            def h_block(t, s):
                ps = ps_par[t % 2]
                for g in range(4):
                    for q in range(4):
                        nc.tensor.matmul(
                            ps[32 * q:32 * q + 32, g, SB * s:SB * s + SB],
                            wh[32 * q:32 * q + 32, g, 32 * q:32 * q + 32],
                            h_st[s][32 * q:32 * q + 32, :],
                            start=False, stop=(s == N_STREAM - 1),
                            tile_position=(32 * q, 32 * q),
                            skip_group_check=True)

            def cell_update(t):
                ps = ps_par[t % 2]
                for s in range(N_STREAM):
                    # on-chain sigmoids (j, i, f); sig(o) only gates h later
                    nc.scalar.activation(S[s][:, 0:3, :],
                                         ps[:, 0:3, SB * s:SB * s + SB],
                                         AF.Sigmoid)
                    nc.scalar.activation(S[s][:, 3, :],
                                         ps[:, 3, SB * s:SB * s + SB],
                                         AF.Sigmoid)
                for s in range(N_STREAM):
                    # tj = 2*sig(2j) - 1 = tanh(j)   (4x-mode tensor_scalar)
                    nc.vector.tensor_scalar(
                        out=TJ[s][:], in0=S[s][:, 0, :],
                        scalar1=2.0, scalar2=-1.0, op0=OP.mult, op1=OP.add)
                    # u = tanh(j) * sig(i)
                    nc.vector.tensor_tensor(
                        u_t[s][:], TJ[s][:], S[s][:, 1, :], OP.mult)
                    # v = c * f'
                    nc.vector.tensor_tensor(
                        v_t[s][:], c_st[:, s, :], S[s][:, 2, :], OP.mult)
                    # c = u + v
                    nc.vector.tensor_tensor(
                        c_st[:, s, :], u_t[s][:], v_t[s][:], OP.add)
                    nc.scalar.activation(tanh_c[s][:], c_st[:, s, :], AF.Tanh)
                for s in range(N_STREAM):
                    nc.vector.tensor_tensor(
                        h_st[s][:], tanh_c[s][:], S[s][:, 3, :], OP.mult)

            xblks = []
            for tb in range(N_BLK):
                xblk = xpool.tile([128, T_BLK, 2, N_STREAM, SB], f16,
                                  tag="xblk")
                nc.sync.dma_start(out=xblk[:], in_=x_in[:, tb])
                xblks.append(xblk)

            for t in range(T):
                x_block(t, xblks[t // T_BLK], [0, 1, 2, 3])
                if t > 0:
                    h_block(t, 0)
                    h_block(t, 1)
                cell_update(t)

            # ---- dense head + ELU per stream ----
            for s in range(N_STREAM):
                y_ps = ps_par[s][0:4, 0, 0:SB]
                nc.tensor.matmul(y_ps, wd[:], h_st[s][:],
                                 start=True, stop=True,
                                 tile_position=(0, 0), skip_group_check=True)
                ybd = state.tile([4, SB], f32)
                m0 = state.tile([4, SB], f32)
                ex = state.tile([4, SB], f32)
                elu = state.tile([4, SB], f32)
                nc.vector.tensor_scalar_add(ybd[:], y_ps, float(bd_val))
                nc.vector.tensor_scalar_min(m0[:], ybd[:], 0.0)
                nc.scalar.activation(ex[:], m0[:], AF.Exp)
                nc.vector.scalar_tensor_tensor(
                    elu[:], ex[:], 1.0, ybd[:], OP.subtract, OP.max)
                nc.sync.dma_start(out=out_ext[s], in_=elu[:])
            stk.close()

    nc.compile()
    return nc


def _prep_weights(W_lstm, b_lstm, W_dense, b_dense):
    Wx = W_lstm[:F, :].astype(np.float32).copy()   # [64, 128]
    Wh = W_lstm[F:, :].astype(np.float32).copy()   # [32, 128]

    # bank order [j, i, f, o] (ref col order i, j, f, o); j folded by 2 so
    # tanh(j) = 2*sig(2j) - 1
    bank_src = (1, 0, 2, 3)
    Wx[:, 32:64] *= 2.0
    Wh[:, 32:64] *= 2.0
    wx_host = np.zeros((128, 4, 128), np.float32)
    wh_host = np.zeros((128, 4, 128), np.float32)
    for g in range(4):
        gs = bank_src[g]
        for q in range(4):
            a = q % 2
            wx_host[64 * a:64 * a + 64, g, 32 * q:32 * q + 32] = \
                Wx[:, 32 * gs:32 * gs + 32]
            wh_host[32 * q:32 * q + 32, g, 32 * q:32 * q + 32] = \
                Wh[:, 32 * gs:32 * gs + 32]

    b = b_lstm.astype(np.float32).copy()
    b_g = [b[32 * g:32 * g + 32].copy() for g in bank_src]
    b_g[0] *= 2.0
    b_g[2] += FORGET_BIAS
    bias_host = np.zeros((1, 4, 128), np.float32)
    for g in range(4):
        bias_host[0, g, :] = np.tile(b_g[g], 4)

    ones_host = np.ones((1, N_STREAM * SB), np.float32)
    wd_host = np.zeros((128, 4), np.float32)
    for q in range(4):
        wd_host[32 * q:32 * q + 32, q] = W_dense[:, 0]
    bd_host = np.array([[np.float32(b_dense.reshape(-1)[0])]], np.float32)
    return (wx_host.astype(np.float16), wh_host.astype(np.float16),
            bias_host.astype(np.float16), ones_host.astype(np.float16),
            wd_host.astype(np.float16), bd_host)


def kernel(x, W_lstm, b_lstm, W_dense, b_dense):
    from concourse.bass_utils import run_bass_kernel_spmd

    x = np.asarray(x, np.float32)
    key = "k"
    if key not in _CACHE:
        _CACHE[key] = _build_kernel(
            np.asarray(b_lstm, np.float32),
            float(np.asarray(b_dense).reshape(-1)[0]))
    nc = _CACHE[key]

    wx, wh, bias_h, ones_h, wd, bd = _prep_weights(
        np.asarray(W_lstm, np.float32), np.asarray(b_lstm, np.float32),
        np.asarray(W_dense, np.float32), np.asarray(b_dense, np.float32))

    in_maps = []
    for c in range(N_CORES):
        xs = x[c * B_LOC:(c + 1) * B_LOC]  # [2048, 16384]
        # [s, j, a, b, tb, ti, f] -> [a, f, tb, ti, j, s, b]
        x7 = xs.reshape(N_STREAM, 2, 2, SB, N_BLK, T_BLK, F)
        x7 = np.ascontiguousarray(
            x7.transpose(2, 6, 4, 5, 1, 0, 3)).astype(np.float16)
        x6 = x7.reshape(128, N_BLK, T_BLK, 2, N_STREAM, SB)
        in_maps.append({
            "x_in": x6, "wx_in": wx, "wh_in": wh, "bias_in": bias_h,
            "ones_in": ones_h, "wd_in": wd,
        })

    res = run_bass_kernel_spmd(nc, in_maps, core_ids=list(range(N_CORES)),
                               tmpdir=os.environ.get("BASS_TMPDIR") or None)
    global LAST_EXEC_NS, LAST_RESULT
    LAST_EXEC_NS = res.exec_time_ns
    LAST_RESULT = res
    outs = [r["out_ext"].reshape(-1) for r in res.results]
    return np.concatenate(outs).astype(np.float32)


LAST_EXEC_NS = None
LAST_RESULT = None


# revision 42
# speedup vs baseline: 1.1751x; 1.0197x over previous
"""Trainium2 Bass kernel for nn_LstmModel (TF-style LSTM, T=256, F=64, H=32,
dense(1)+ELU head), data-parallel over 8 NeuronCores.

v2 design (per core, B_loc = 2048 rows):
  - x is transposed + cast to fp16 on host: no on-chip transpose pass.
  - 2 independent batch streams of 1024 rows each; per stream the state is
    chunk-packed [128 = 4 subchunks x 32 h, 256 batch].
  - gates PSUM tile per stream [128, 4 gates, 256] (2 banks), double-buffered.
  - per step per stream:
      PE: rank-1 bias matmul seeds the f-gate slice with (b_f + 1); X-pass is
          4 gates x 2 col-tiled matmuls (rhs [128,256] = 2 subchunks); H-pass
          is 4 block-diagonal full-array matmuls vs h [128,256].
      ACT: one Sigmoid over the whole [128, 1024] gate tile (j cols pre-scaled
          by 2 so tanh(j) = 2*sig(2j)-1), one Tanh over c (merged across
          streams, [128,512]).
      DVE: u=(sig2j-0.5)*sigi; v=c*f'; c=2u+v; h=tanh_c*o'   (all fp16 SBUF)
  - tail: block-diag dense matmul + ELU per stream.
"""

import os
import sys

import numpy as np

sys.path.insert(0, "/opt/trn_rl_repo")

# ---- problem constants (hardcoded per harness contract) ----
B_FULL = 16384
T = 256
F = 64
H = 32
FORGET_BIAS = 1.0
N_CORES = 8
B_LOC = B_FULL // N_CORES          # 2048
N_STREAM = 2
SB = 256                           # batch per subchunk (free dim)
T_BLK = 16                         # time steps per x DMA block
N_BLK = T // T_BLK                 # 16 blocks

_CACHE = {}


def _build_kernel(b_lstm_host, bd_val):
    import concourse.bass as bass  # noqa: F401
    import concourse.tile as tile
    from concourse import bacc, mybir

    f32 = mybir.dt.float32
    f16 = mybir.dt.float16
    AF = mybir.ActivationFunctionType
    OP = mybir.AluOpType

    b = b_lstm_host.astype(np.float32)
    # bank order [j, i, f, o]; reference gate order is i, j, f, o
    b_g = [b[32 * g:32 * g + 32].copy() for g in (1, 0, 2, 3)]
    b_g[0] *= 2.0
    b_g[2] += FORGET_BIAS
    need_bias = [bool(np.any(b_g[g] != 0.0)) for g in range(4)]

    nc = bacc.Bacc(None, target_bir_lowering=False, debug=False)

    with tile.TileContext(nc) as tc:
        with tc.tile_pool(name="dram", bufs=1, space="DRAM") as dram:
            # x pre-arranged on host to [p=64a+f, tb, ti, j, s, b]
            x_in = dram.tile([128, N_BLK, T_BLK, 2, N_STREAM, SB], f16,
                             kind="ExternalInput", name="x_in", uniquify=False)
            wx_in = dram.tile([128, 4, 128], f16, kind="ExternalInput",
                              name="wx_in", uniquify=False)
            wh_in = dram.tile([128, 4, 128], f16, kind="ExternalInput",
                              name="wh_in", uniquify=False)
            bias_in = dram.tile([1, 4, 128], f16, kind="ExternalInput",
                                name="bias_in", uniquify=False)
            ones_in = dram.tile([1, N_STREAM * SB], f16, kind="ExternalInput",
                                name="ones_in", uniquify=False)
            wd_in = dram.tile([128, 4], f16, kind="ExternalInput",
                              name="wd_in", uniquify=False)
            out_ext = dram.tile([N_STREAM, 4, SB], f32, kind="ExternalOutput",
                                name="out_ext", uniquify=False)

            from contextlib import ExitStack
            stk = ExitStack()
            const = stk.enter_context(tc.tile_pool(name="const", bufs=1))
            wx = const.tile([128, 4, 128], f16)
            wh = const.tile([128, 4, 128], f16)
            bias_t = const.tile([1, 4, 128], f16)
            onesr = const.tile([1, N_STREAM * SB], f16)
            wd = const.tile([128, 4], f16)
            nc.sync.dma_start(out=wx[:], in_=wx_in[:])
            nc.sync.dma_start(out=wh[:], in_=wh_in[:])
            nc.sync.dma_start(out=bias_t[:], in_=bias_in[:])
            nc.sync.dma_start(out=onesr[:], in_=ones_in[:])
            nc.sync.dma_start(out=wd[:], in_=wd_in[:])

            # persistent state (both streams side by side where useful)
            state = stk.enter_context(tc.tile_pool(name="state", bufs=1))
            c_st = state.tile([128, N_STREAM, SB], f16)
            tanh_c = [state.tile([128, SB], f16, name=f"tanh_c{s}")
                      for s in range(N_STREAM)]
            h_st = [state.tile([128, SB], f16, name=f"h_st{s}")
                    for s in range(N_STREAM)]
            # bank order is [j, i, f, o]; S holds sig over all four banks
            TJ = [state.tile([128, SB], f16, name=f"TJ{s}")
                  for s in range(N_STREAM)]
            S = [state.tile([128, 4, SB], f16, name=f"S{s}")
                 for s in range(N_STREAM)]
            u_t = [state.tile([128, SB], f16, name=f"u_t{s}")
                   for s in range(N_STREAM)]
            v_t = [state.tile([128, SB], f16, name=f"v_t{s}")
                   for s in range(N_STREAM)]

            nc.vector.memset(c_st[:], 0.0)

            psum_p = stk.enter_context(
                tc.tile_pool(name="psp", bufs=1, space="PSUM"))
            # parity-alternated gate tiles, both streams side by side in the
            # free dim: [128 = 4 subchunks x 32h, 4 gates (1 bank each),
            # 512 = 2 streams x 256 batch]
            ps_par = [psum_p.tile([128, 4, N_STREAM * SB], f32,
                                  name=f"ps_par{p}") for p in range(2)]
            xpool = stk.enter_context(tc.tile_pool(name="xpool", bufs=2))

            def x_block(t, xblk, gates):
                """bias + X-pass rect matmuls for both streams (N=512)."""
                ps = ps_par[t % 2]
                ti = t % T_BLK
                last_x = (t == 0)  # no H contribution at t=0
                for g in gates:
                    if need_bias[g]:
                        nc.tensor.matmul(
                            ps[:, g, :], bias_t[0:1, g, :], onesr[0:1, :],
                            start=True, stop=False,
                            tile_position=(0, 0), skip_group_check=True)
                    for q in range(4):
                        a, j = q % 2, q // 2
                        nc.tensor.matmul(
                            ps[32 * q:32 * q + 32, g, :],
                            wx[64 * a:64 * a + 64, g, 32 * q:32 * q + 32],
                            xblk[64 * a:64 * a + 64, ti, j, :, :],
                            start=not need_bias[g], stop=last_x,
                            tile_position=(64 * a, 32 * q),
                            skip_group_check=True)

            def h_block(t, s):
                ps = ps_par[t % 2]
                for g in range(4):
                    for q in range(4):
                        nc.tensor.matmul(
                            ps[32 * q:32 * q + 32, g, SB * s:SB * s + SB],
                            wh[32 * q:32 * q + 32, g, 32 * q:32 * q + 32],
                            h_st[s][32 * q:32 * q + 32, :],
                            start=False, stop=(s == N_STREAM - 1),
                            tile_position=(32 * q, 32 * q),
                            skip_group_check=True)

            def cell_update(t):
                ps = ps_par[t % 2]
                for s in range(N_STREAM):
                    nc.scalar.activation(S[s][:, :, :],
                                         ps[:, :, SB * s:SB * s + SB],
                                         AF.Sigmoid)
                for s in range(N_STREAM):
                    # tj = 2*sig(2j) - 1 = tanh(j)   (4x-mode tensor_scalar)
                    nc.vector.tensor_scalar(
                        out=TJ[s][:], in0=S[s][:, 0, :],
                        scalar1=2.0, scalar2=-1.0, op0=OP.mult, op1=OP.add)
                    # u = tanh(j) * sig(i)
                    nc.vector.tensor_tensor(
                        u_t[s][:], TJ[s][:], S[s][:, 1, :], OP.mult)
                    # v = c * f'
                    nc.vector.tensor_tensor(
                        v_t[s][:], c_st[:, s, :], S[s][:, 2, :], OP.mult)
                    # c = u + v
                    nc.vector.tensor_tensor(
                        c_st[:, s, :], u_t[s][:], v_t[s][:], OP.add)
                    nc.scalar.activation(tanh_c[s][:], c_st[:, s, :], AF.Tanh)
                for s in range(N_STREAM):
                    nc.vector.tensor_tensor(
                        h_st[s][:], tanh_c[s][:], S[s][:, 3, :], OP.mult)

            xblks = []
            for tb in range(N_BLK):
                xblk = xpool.tile([128, T_BLK, 2, N_STREAM, SB], f16,
                                  tag="xblk")
                nc.sync.dma_start(out=xblk[:], in_=x_in[:, tb])
                xblks.append(xblk)

            # dense matmul burst to push the PE HAM clock gate to 8/8
            # (~4us of sustained activity); results are discarded
            for k in range(24):
                nc.tensor.matmul(
                    ps_par[0][:, k % 4, :], wx[:, 0, :], wx[:, :, :],
                    start=True, stop=True, tile_position=(0, 0),
                    skip_group_check=True)

            for t in range(T):
                x_block(t, xblks[t // T_BLK], [0, 1, 2, 3])
                if t > 0:
                    h_block(t, 0)
                    h_block(t, 1)
                cell_update(t)

            # ---- dense head + ELU per stream ----
            for s in range(N_STREAM):
                y_ps = ps_par[s][0:4, 0, 0:SB]
                nc.tensor.matmul(y_ps, wd[:], h_st[s][:],
                                 start=True, stop=True,
                                 tile_position=(0, 0), skip_group_check=True)
                ybd = state.tile([4, SB], f32)
                m0 = state.tile([4, SB], f32)
                ex = state.tile([4, SB], f32)
                elu = state.tile([4, SB], f32)
                nc.vector.tensor_scalar_add(ybd[:], y_ps, float(bd_val))
                nc.vector.tensor_scalar_min(m0[:], ybd[:], 0.0)
                nc.scalar.activation(ex[:], m0[:], AF.Exp)
                nc.vector.scalar_tensor_tensor(
                    elu[:], ex[:], 1.0, ybd[:], OP.subtract, OP.max)
                nc.sync.dma_start(out=out_ext[s], in_=elu[:])
            stk.close()

    nc.compile()
    return nc


def _prep_weights(W_lstm, b_lstm, W_dense, b_dense):
    Wx = W_lstm[:F, :].astype(np.float32).copy()   # [64, 128]
    Wh = W_lstm[F:, :].astype(np.float32).copy()   # [32, 128]

    # bank order [j, i, f, o] (ref col order i, j, f, o); j folded by 2 so
    # tanh(j) = 2*sig(2j) - 1
    bank_src = (1, 0, 2, 3)
    Wx[:, 32:64] *= 2.0
    Wh[:, 32:64] *= 2.0
    wx_host = np.zeros((128, 4, 128), np.float32)
    wh_host = np.zeros((128, 4, 128), np.float32)
    for g in range(4):
        gs = bank_src[g]
        for q in range(4):
            a = q % 2
            wx_host[64 * a:64 * a + 64, g, 32 * q:32 * q + 32] = \
                Wx[:, 32 * gs:32 * gs + 32]
            wh_host[32 * q:32 * q + 32, g, 32 * q:32 * q + 32] = \
                Wh[:, 32 * gs:32 * gs + 32]

    b = b_lstm.astype(np.float32).copy()
    b_g = [b[32 * g:32 * g + 32].copy() for g in bank_src]
    b_g[0] *= 2.0
    b_g[2] += FORGET_BIAS
    bias_host = np.zeros((1, 4, 128), np.float32)
    for g in range(4):
        bias_host[0, g, :] = np.tile(b_g[g], 4)

    ones_host = np.ones((1, N_STREAM * SB), np.float32)
    wd_host = np.zeros((128, 4), np.float32)
    for q in range(4):
        wd_host[32 * q:32 * q + 32, q] = W_dense[:, 0]
    bd_host = np.array([[np.float32(b_dense.reshape(-1)[0])]], np.float32)
    return (wx_host.astype(np.float16), wh_host.astype(np.float16),
            bias_host.astype(np.float16), ones_host.astype(np.float16),
            wd_host.astype(np.float16), bd_host)


def kernel(x, W_lstm, b_lstm, W_dense, b_dense):
    from concourse.bass_utils import run_bass_kernel_spmd

    x = np.asarray(x, np.float32)
    key = "k"
    if key not in _CACHE:
        _CACHE[key] = _build_kernel(
            np.asarray(b_lstm, np.float32),
            float(np.asarray(b_dense).reshape(-1)[0]))
    nc = _CACHE[key]

    wx, wh, bias_h, ones_h, wd, bd = _prep_weights(
        np.asarray(W_lstm, np.float32), np.asarray(b_lstm, np.float32),
        np.asarray(W_dense, np.float32), np.asarray(b_dense, np.float32))

    in_maps = []
    for c in range(N_CORES):
        xs = x[c * B_LOC:(c + 1) * B_LOC]  # [2048, 16384]
        # [s, j, a, b, tb, ti, f] -> [a, f, tb, ti, j, s, b]
        x7 = xs.reshape(N_STREAM, 2, 2, SB, N_BLK, T_BLK, F)
        x7 = np.ascontiguousarray(
            x7.transpose(2, 6, 4, 5, 1, 0, 3)).astype(np.float16)
        x6 = x7.reshape(128, N_BLK, T_BLK, 2, N_STREAM, SB)
        in_maps.append({
            "x_in": x6, "wx_in": wx, "wh_in": wh, "bias_in": bias_h,
            "ones_in": ones_h, "wd_in": wd,
        })

    res = run_bass_kernel_spmd(nc, in_maps, core_ids=list(range(N_CORES)),
                               tmpdir=os.environ.get("BASS_TMPDIR") or None)
    global LAST_EXEC_NS, LAST_RESULT
    LAST_EXEC_NS = res.exec_time_ns
    LAST_RESULT = res
    outs = [r["out_ext"].reshape(-1) for r in res.results]
    return np.concatenate(outs).astype(np.float32)


LAST_EXEC_NS = None
LAST_RESULT = None
